# revision 40
# baseline (speedup 1.0000x reference)
"""ContextualAttention Trainium2 kernel.

Full (unsharded) inputs in, full output out. Internally shards across 8
NeuronCores as batch x head-group: core c -> batch c//2, heads
(c%2)*8 .. (c%2)*8+8.  Each core computes a partial output projection for
its batch; the host sums the two partials per batch and adds the bias.

Per-core dataflow (per-core shapes; E=1024, heads=8 local, d=64):
  Q/K projections run in fp8e4 with DoubleRow perf mode (contraction 256
  per matmul), with the QK-layernorm centering matrix folded into the
  weights on the host.  Variance via block-ones matmul on squared raws;
  1/sigma = Exp(-0.5 * Ln(var + eps)) so the whole LN chain stays in the
  exp activation-table set (no table switches).  gamma/beta and the
  1/E**0.25 scaling folded into a fused tensor_scalar.
  V projection, S->P exp output, PV, and unify all run in bf16.
  S^T tiles = K_hT.T @ Q_hT (f32..bf16 lhsT), both heads of a pair into
  one 2-bank psum tile -> a single 1024-wide Exp per (pair, jt).
  [out_h^T; denom] = [V_h | 1].T @ P accumulated over j tiles.
  Denominator reciprocal broadcast across partitions with
  gpsimd.partition_broadcast (no DRAM round-trip).
  y_part = outT.T @ WuT-slices in bf16 -> [1024, 1024] fp32 psum.
"""

import sys

import numpy as np
import ml_dtypes

sys.path.insert(0, "/opt/trn_rl_repo")

import concourse.bass as bass  # noqa: E402
from concourse import bacc  # noqa: E402
import concourse.tile as tile  # noqa: E402
from concourse import mybir  # noqa: E402
from concourse.bass_utils import run_bass_kernel_spmd  # noqa: E402

F32 = mybir.dt.float32
F32R = mybir.dt.float32r
BF16 = mybir.dt.bfloat16
FP8 = mybir.dt.float8e4
AF = mybir.ActivationFunctionType
ALU = mybir.AluOpType
DRMODE = mybir.MatmulPerfMode.DoubleRow

# All ACT functions this kernel uses (Square, Ln, Exp, Copy) live together in
# the natural_log_exp_and_others table set, but the table-load pass assigns
# each function the first set that contains it, which alternates table loads
# (~1.3us each) between sets inside the LN pipeline.  Steer the pass by
# hiding these functions from every other set; canonical set indices are
# preserved so walrus/runtime still agree on ids.
import concourse.hw_specs as _hw_specs  # noqa: E402

_ORIG_GAT = _hw_specs.get_activation_tables
_ONE_SET = "natural_log_exp_and_others"
_SHARED_FNS = {AF.Exp, AF.Square, AF.Copy, AF.Ln, AF.Identity}


def _gat_one_set(arch):
    tables = _ORIG_GAT(arch)
    return {
        name: (funcs if name == _ONE_SET else funcs - _SHARED_FNS)
        for name, funcs in tables.items()
    }


bacc.get_activation_tables = _gat_one_set

E = 1024
TI = 1024
T = 2048
HL = 8  # heads per core
D = 64  # head size
DL = HL * D  # 512, local q/k/v dim
B = 4
SCALE = float(E) ** 0.25
EPS = 1e-5

import os

# DVE polynomial-exp offload: measured slower than the ACT-only build (the
# extra DVE traffic and single-buffered PV psum cost more than the offloaded
# ACT time saved), so off by default.
K_POLY = os.environ.get("K_POLY", "0") == "1"

# exp(x) ~= EC[7]*M(x) + EC[0] with monic chain M(x) = (..((x+m6)x+m5*?)..)
# Chebyshev deg-7 fit on [-1.4, 1.4] (LN'd scores are within +-1.25); the
# chain h=(h+m_k)*x maps to scalar_tensor_tensor, 2-byte eligible on DVE.
EC = [0.99999834, 0.99999913, 0.50003212, 0.16667357,
      0.041571689, 0.0083188626, 0.0014769934, 0.00020948640]
EM = [c / EC[7] for c in EC]  # EM[1..6] used


def _emit(tc, xc8, xcb, wq8, wk8, wvb, wub, lnob, gq, bq, gk, bk, y):
    nc = tc.nc
    from contextlib import ExitStack

    with ExitStack() as ctx:
        consts = ctx.enter_context(tc.tile_pool(name="consts", bufs=1))
        resid = ctx.enter_context(tc.tile_pool(name="resid", bufs=1))

        # ---- residents (live through all phases) ----
        qt_sb = resid.tile([128, 4, TI], BF16, tag="qt")  # [2-head pair, pair#, i]
        kt_sb = resid.tile([128, 4, T], BF16, tag="kt")
        v_sb = resid.tile([128, 16, HL * (D + 1)], BF16, tag="v")  # +ones col

        # ones column at col D of each head slot ([V | 1])
        v_heads = v_sb.rearrange("p j (h e) -> p j h e", e=D + 1)
        nc.vector.memset(v_heads[:, :, :, D : D + 1], 1.0)

        # wu resident; DMA'd early on the gpsimd queue (only used in phase U)
        wu_sb = resid.tile([128, 4, E], BF16, tag="wu")
        wuT_r = wub.rearrange("(ko p) e -> p ko e", p=128)

        # ---- phase P: projections + LN (xc streamed in 512-token blocks) ----
        with (
            tc.tile_pool(name="xc8", bufs=2) as xc8_pool,
            tc.tile_pool(name="xcb", bufs=2) as xcb_pool,
            tc.tile_pool(name="w", bufs=1) as w_pool,
            tc.tile_pool(name="ln_tmp", bufs=4) as ln_tmp,
            tc.tile_pool(name="pp", bufs=4, space="PSUM") as pp,
            tc.tile_pool(name="pstat", bufs=2, space="PSUM") as pstat,
        ):
            xc8_r = xc8.rearrange("(ko p) t -> p ko t", p=128)
            xcb_r = xcb.rearrange("(ko p) t -> p ko t", p=128)

            xc8_tiles = {}
            xcb_tiles = {}

            def fetch_xc8(nt):
                t8 = xc8_pool.tile([128, 8, 512], FP8, tag="xc8", name="xc8_nt")
                tsl = slice(nt * 512, (nt + 1) * 512)
                if nt == 0:
                    # split so the first projection groups gate on the first
                    # half only (finer write->read region granularity)
                    nc.sync.dma_start(out=t8[:, 0:4], in_=xc8_r[:, 0:4, tsl])
                    nc.sync.dma_start(out=t8[:, 4:8], in_=xc8_r[:, 4:8, tsl])
                else:
                    nc.sync.dma_start(out=t8, in_=xc8_r[:, :, tsl])
                xc8_tiles[nt] = t8

            def fetch_xcb(nt):
                tb = xcb_pool.tile([128, 8, 512], BF16, tag="xcb", name="xcb_nt")
                nc.sync.dma_start(out=tb, in_=xcb_r[:, :, nt * 512 : (nt + 1) * 512])
                xcb_tiles[nt] = tb

            # DMA order is critical-path order: the first Q projection group
            # needs only xc8[0] (sync queue) + wq8 (gpsimd queue); everything
            # else queues behind those.
            w_t = {}
            w_t["q"] = w_pool.tile([128, 8, DL], FP8, tag="wq", name="wq")
            wq8_r = wq8.rearrange("(ko p) m -> p ko m", p=128)
            # split so the first projection group gates on the first half
            nc.gpsimd.dma_start(out=w_t["q"][:, 0:4], in_=wq8_r[:, 0:4])
            nc.gpsimd.dma_start(out=w_t["q"][:, 4:8], in_=wq8_r[:, 4:8])
            fetch_xc8(0)
            w_t["k"] = w_pool.tile([128, 8, DL], FP8, tag="wk", name="wk")
            nc.gpsimd.dma_start(
                out=w_t["k"], in_=wk8.rearrange("(ko p) m -> p ko m", p=128)
            )
            w_t["v"] = w_pool.tile([128, 8, DL], BF16, tag="wv", name="wv")
            nc.gpsimd.dma_start(
                out=w_t["v"], in_=wvb.rearrange("(ko p) m -> p ko m", p=128)
            )
            fetch_xcb(0)
            # consts (needed by the first LN stage2, a few groups in) queue
            # behind the first xc/xcb blocks on sync
            lno_sb = consts.tile([128, 128], BF16, tag="lno")
            nc.sync.dma_start(out=lno_sb, in_=lnob)
            g_sb = {}
            b_sb = {}
            for name, src in (("q", gq), ("k", gk)):
                g_sb[name] = consts.tile([128, 1], F32, tag=f"g{name}", name=f"g{name}")
                nc.sync.dma_start(out=g_sb[name], in_=src)
            for name, src in (("q", bq), ("k", bk)):
                b_sb[name] = consts.tile([128, 1], F32, tag=f"b{name}", name=f"b{name}")
                nc.sync.dma_start(out=b_sb[name], in_=src)
            eps_sb = consts.tile([128, 1], F32, tag="eps")
            nc.vector.memset(eps_sb, EPS)

            fetch_xc8(1)
            fetch_xcb(1)
            for k in range(4):
                nc.gpsimd.dma_start(out=wu_sb[:, k], in_=wuT_r[:, k])

            # deferred-LN pipeline: stage1 (Square) runs one matmul-group
            # after the projection, stage2 (mvar matmul + Ln/Exp rsqrt + the
            # normalize muls) two groups after, so the PE never waits on
            # scalar/vector latency.
            pipe = []

            def ln_stage1(it):
                sq = ln_tmp.tile([128, 512], BF16, tag="sq", name="sq")
                nc.scalar.activation(sq, it["ps"], AF.Square)
                it["sq"] = sq

            def ln_stage2(it):
                mvar = pstat.tile([128, 512], F32, tag="mvar", name="mvar")
                nc.tensor.matmul(
                    mvar, lhsT=lno_sb, rhs=it["sq"], start=True, stop=True
                )
                lnv = ln_tmp.tile([128, 512], F32, tag="lnv", name="lnv")
                nc.scalar.activation(lnv, mvar, AF.Ln, bias=eps_sb)
                rs = ln_tmp.tile([128, 512], F32, tag="rs", name="rs")
                nc.scalar.activation(rs, lnv, AF.Exp, scale=-0.5)
                nc.vector.tensor_mul(it["dest"], it["ps"], rs)
                nc.vector.tensor_scalar(
                    it["dest"], it["dest"], it["gs"], it["bs"], ALU.mult, ALU.add
                )

            def advance():
                if pipe and pipe[0]["stage"] == 1:
                    ln_stage2(pipe.pop(0))
                if pipe and pipe[0]["stage"] == 0:
                    ln_stage1(pipe[0])
                    pipe[0]["stage"] = 1

            for nt in range(4):  # 512-token blocks of xc
                if nt + 2 < 4:
                    fetch_xc8(nt + 2)
                    fetch_xcb(nt + 2)
                xc8_nt = xc8_tiles.pop(nt)
                xcb_nt = xcb_tiles.pop(nt)

                for mc in range(4):
                    # Q projection covers only the first TI tokens
                    projs = [("k", kt_sb)] if nt >= 2 else [("q", qt_sb), ("k", kt_sb)]
                    for wname, dest in projs:
                        ps = pp.tile([128, 512], F32, tag="pp", name="ps")
                        wt = w_t[wname].rearrange("p (ko t) m -> p ko t m", t=2)
                        x8 = xc8_nt.rearrange("p (ko t) n -> p ko t n", t=2)
                        for ko in range(4):
                            nc.tensor.matmul(
                                ps,
                                lhsT=wt[:, ko, :, mc * 128 : (mc + 1) * 128],
                                rhs=x8[:, ko],
                                start=(ko == 0),
                                stop=(ko == 3),
                                perf_mode=DRMODE,
                            )
                        pipe.append(
                            {
                                "stage": 0,
                                "ps": ps,
                                "dest": dest[:, mc, nt * 512 : (nt + 1) * 512],
                                "gs": g_sb[wname],
                                "bs": b_sb[wname],
                            }
                        )
                        advance()
                    # V natural projection tile interleaved to pad the PE
                    # stream while the LN chain of the Q/K groups completes
                    tt = nt * 4 + mc
                    ps = pp.tile([128, 512], F32, tag="pp", name="ps")
                    for k in range(8):
                        nc.tensor.matmul(
                            ps,
                            lhsT=xcb_nt[:, k, mc * 128 : (mc + 1) * 128],
                            rhs=w_t["v"][:, k, :],
                            start=(k == 0),
                            stop=(k == 7),
                        )
                    nc.vector.tensor_copy(
                        v_heads[:, tt, :, 0:D],
                        ps.rearrange("p (h e) -> p h e", e=D),
                    )
                    advance()
            while pipe:
                advance()

        # ---- phase A: attention ----
        resid2 = ctx.enter_context(tc.tile_pool(name="resid2", bufs=1))
        ot_sb = resid2.tile([128, 4, TI], BF16, tag="ot")

        poly_ctx = ExitStack()
        poly_pool = None
        pqp = None
        if K_POLY:
            poly_pool = poly_ctx.enter_context(tc.tile_pool(name="poly", bufs=2))
            pqp = poly_ctx.enter_context(
                tc.tile_pool(name="pqp", bufs=1, space="PSUM")
            )

        def poly_chain(eng, pt, xb, sfx):
            # exp(x) ~= EC7*((..((x+m6)x+m5)x..)x) + EC0 via stt chain
            h = poly_pool.tile([128, 1024], BF16, tag=f"h{sfx}", name=f"h{sfx}")
            eng.scalar_tensor_tensor(h, xb, EM[6], xb, ALU.add, ALU.mult)
            for k in (5, 4, 3, 2, 1):
                eng.scalar_tensor_tensor(h, h, EM[k], xb, ALU.add, ALU.mult)
            eng.tensor_scalar(pt, h, EC[7], EC[0], ALU.mult, ALU.add)

        with (
            tc.tile_pool(name="pt", bufs=4) as pt_pool,
            tc.tile_pool(name="sm", bufs=2) as sm_pool,
            tc.tile_pool(name="pqk", bufs=2, space="PSUM") as pqk,
            tc.tile_pool(name="ppv", bufs=1 if K_POLY else 2, space="PSUM") as ppv,
        ):
            JT_GP, JT_DVE = (15, 4) if K_POLY else (None, None)
            for hp in range(4):
                for ic in range(2):
                    isl = slice(ic * 512, (ic + 1) * 512)
                    # pv{0,1}: [out_h^T; denom_h] rows 0:65
                    pv0 = ppv.tile([128, 512], F32, tag="pv0", name="pv0")
                    pv1 = ppv.tile([128, 512], F32, tag="pv1", name="pv1")
                    h0, h1 = 2 * hp, 2 * hp + 1

                    def emit_qk(jt, tag):
                        jsl = slice(jt * 128, (jt + 1) * 128)
                        pool = pqp if tag == "pqp" else pqk
                        pq = pool.tile([128, 1024], F32, tag=tag, name=tag)
                        nc.tensor.matmul(
                            pq[:, 0:512],
                            lhsT=kt_sb[0:64, hp, jsl],
                            rhs=qt_sb[0:64, hp, isl],
                            start=True,
                            stop=True,
                        )
                        nc.tensor.matmul(
                            pq[:, 512:1024],
                            lhsT=kt_sb[64:128, hp, jsl],
                            rhs=qt_sb[64:128, hp, isl],
                            start=True,
                            stop=True,
                        )
                        return pq

                    pv_first = {"v": True}

                    def emit_pv(jt, pt, last=False):
                        st = pv_first["v"]
                        pv_first["v"] = False
                        nc.tensor.matmul(
                            pv0[0:65],
                            lhsT=v_sb[:, jt, h0 * (D + 1) : (h0 + 1) * (D + 1)],
                            rhs=pt[:, 0:512],
                            start=st,
                            stop=last,
                        )
                        nc.tensor.matmul(
                            pv1[0:65],
                            lhsT=v_sb[:, jt, h1 * (D + 1) : (h1 + 1) * (D + 1)],
                            rhs=pt[:, 512:1024],
                            start=st,
                            stop=last,
                        )

                    pt_poly = {}
                    if K_POLY:
                        # one poly-exp tile on DVE: its QK runs first into the
                        # dedicated pqp psum slot (freed by the copy, never
                        # contended), the chain runs while ACT does the rest
                        pq_g = emit_qk(JT_GP, "pqp")
                        pt_g = pt_pool.tile([128, 1024], BF16, tag="ptg", name="ptg")
                        xb_g = poly_pool.tile([128, 1024], BF16, tag="xbg", name="xbg")
                        nc.vector.tensor_copy(xb_g, pq_g)
                        poly_chain(nc.vector, pt_g, xb_g, "g")
                        pt_poly[JT_GP] = pt_g

                    act_jts = [j for j in range(16) if j not in pt_poly]
                    for n, jt in enumerate(act_jts):
                        pq = emit_qk(jt, "pq")
                        pt = pt_pool.tile([128, 1024], BF16, tag="pt", name="pt")
                        nc.scalar.activation(pt, pq, AF.Exp)
                        emit_pv(jt, pt, last=(not K_POLY and n == len(act_jts) - 1))
                    if K_POLY:
                        emit_pv(JT_GP, pt_poly[JT_GP], last=True)
                    # normalize (overlapped with the next pair's matmuls)
                    for par, pv in ((0, pv0), (1, pv1)):
                        rc = sm_pool.tile(
                            [65, 512], F32, tag=f"rc{par}", name=f"rc{par}"
                        )
                        # approx recip needs a full-range partition-0 start on
                        # HW; run it over all 65 rows (lanes are parallel,
                        # same cost) and use only row 64
                        den = sm_pool.tile(
                            [65, 512], F32, tag=f"den{par}", name=f"den{par}"
                        )
                        nc.vector.tensor_copy(den, pv[0:65])
                        nc.vector.reciprocal_approx_fast(rc[0:65], den)
                        # partition_broadcast reads absolute partition 0, so
                        # stage the denominator row there first (gpsimd queue
                        # keeps the sync queue free for xc/y traffic)
                        rc0 = sm_pool.tile([1, 512], F32, tag=f"rc0{par}", name=f"rc0{par}")
                        nc.gpsimd.dma_start(out=rc0, in_=rc[64:65, :])
                        bc = sm_pool.tile([64, 512], F32, tag=f"bc{par}", name=f"bc{par}")
                        nc.gpsimd.partition_broadcast(bc, rc0)
                        if par == 0:
                            nc.vector.tensor_mul(
                                ot_sb[0:64, hp, isl], pv[0:64], bc
                            )
                        else:
                            tmp = sm_pool.tile([64, 512], BF16, tag="tmpB", name="tmpB")
                            nc.vector.tensor_mul(tmp, pv[0:64], bc)
                            nc.gpsimd.dma_start(out=ot_sb[64:128, hp, isl], in_=tmp)
        poly_ctx.close()

        # ---- phase U: unify ----
        with (
            tc.tile_pool(name="yp", bufs=3) as y_pool,
            tc.tile_pool(name="pu", bufs=4, space="PSUM") as pu,
        ):
            for it in range(8):
                for et in range(2):
                    py = pu.tile([128, 512], F32, tag="py", name="py")
                    for hp in range(4):
                        nc.tensor.matmul(
                            py,
                            lhsT=ot_sb[:, hp, it * 128 : (it + 1) * 128],
                            rhs=wu_sb[:, hp, et * 512 : (et + 1) * 512],
                            start=(hp == 0),
                            stop=(hp == 3),
                        )
                    ysb = y_pool.tile([128, 512], F32, tag="y", name="ysb")
                    # alternate the psum->sbuf copy between scalar and vector
                    # and the output DMA between sync and gpsimd (idle by
                    # phase U) so neither engine nor queue serializes the tail
                    if (it * 2 + et) % 2 == 0:
                        nc.scalar.activation(ysb, py, AF.Copy)
                        yq = nc.sync
                    else:
                        nc.vector.tensor_copy(ysb, py)
                        yq = nc.gpsimd
                    yq.dma_start(
                        out=y[it * 128 : (it + 1) * 128, et * 512 : (et + 1) * 512],
                        in_=ysb,
                    )


_NC_CACHE = None


def build_nc():
    global _NC_CACHE
    if _NC_CACHE is not None:
        return _NC_CACHE
    nc = bacc.Bacc(
        trn_type="TRN2",
        target_bir_lowering=False,
        debug=False,
        enable_asserts=False,
        num_devices=8,
    )
    xc8 = nc.dram_tensor("xc8", [E, T], FP8, kind="ExternalInput").ap()
    xcb = nc.dram_tensor("xcb", [E, T], BF16, kind="ExternalInput").ap()
    wq8 = nc.dram_tensor("wq8", [E, DL], FP8, kind="ExternalInput").ap()
    wk8 = nc.dram_tensor("wk8", [E, DL], FP8, kind="ExternalInput").ap()
    wvb = nc.dram_tensor("wvb", [E, DL], BF16, kind="ExternalInput").ap()
    wub = nc.dram_tensor("wub", [DL, E], BF16, kind="ExternalInput").ap()
    lnob = nc.dram_tensor("lnob", [128, 128], BF16, kind="ExternalInput").ap()
    gq = nc.dram_tensor("gq", [128, 1], F32, kind="ExternalInput").ap()
    bq = nc.dram_tensor("bq", [128, 1], F32, kind="ExternalInput").ap()
    gk = nc.dram_tensor("gk", [128, 1], F32, kind="ExternalInput").ap()
    bk = nc.dram_tensor("bk", [128, 1], F32, kind="ExternalInput").ap()
    y = nc.dram_tensor("y", [TI, E], F32, kind="ExternalOutput").ap()
    with tile.TileContext(nc) as tc:
        _emit(tc, xc8, xcb, wq8, wk8, wvb, wub, lnob, gq, bq, gk, bk, y)
    nc.compile()
    _NC_CACHE = nc
    return nc


def _prep_inputs(x, context, Wq, Wk, Wv, Wu, q_gamma, q_beta, k_gamma, k_beta):
    f = lambda a: np.ascontiguousarray(np.asarray(a, dtype=np.float32))
    x, context = f(x), f(context)
    Wq, Wk, Wv, Wu = f(Wq), f(Wk), f(Wv), f(Wu)
    lno = np.kron(np.eye(2, dtype=np.float32), np.ones((D, D), np.float32)) / D

    def center_heads(wT):
        # wT: [E, DL]; subtract per-64-column-block mean (folds LN centering)
        w = wT.reshape(E, HL, D)
        return (w - w.mean(axis=2, keepdims=True)).reshape(E, DL)

    consts = {
        "lnob": lno.astype(ml_dtypes.bfloat16),
        "gq": f(np.tile(q_gamma, 2) / SCALE)[:, None],
        "bq": f(np.tile(q_beta, 2) / SCALE)[:, None],
        "gk": f(np.tile(k_gamma, 2) / SCALE)[:, None],
        "bk": f(np.tile(k_beta, 2) / SCALE)[:, None],
    }
    in_maps = []
    for c in range(8):
        b, hh = c // 2, c % 2
        sl = slice(hh * DL, (hh + 1) * DL)
        xc = np.concatenate([x[b], context[b]], axis=0)
        xcT = np.ascontiguousarray(xc.T)
        in_maps.append(
            {
                "xc8": xcT.astype(ml_dtypes.float8_e4m3),
                "xcb": xcT.astype(ml_dtypes.bfloat16),
                "wq8": center_heads(
                    np.ascontiguousarray(Wq[sl].T)
                ).astype(ml_dtypes.float8_e4m3),
                "wk8": center_heads(
                    np.ascontiguousarray(Wk[sl].T)
                ).astype(ml_dtypes.float8_e4m3),
                "wvb": np.ascontiguousarray(Wv[sl].T).astype(ml_dtypes.bfloat16),
                "wub": np.ascontiguousarray(Wu[:, sl].T).astype(ml_dtypes.bfloat16),
                **consts,
            }
        )
    return in_maps


def run(inputs, trace=False):
    bu = np.asarray(inputs["bu"], dtype=np.float32)
    in_maps = _prep_inputs(
        inputs["x"], inputs["context"], inputs["Wq"], inputs["Wk"], inputs["Wv"],
        inputs["Wu"], inputs["q_gamma"], inputs["q_beta"], inputs["k_gamma"],
        inputs["k_beta"],
    )
    nc = build_nc()
    res = run_bass_kernel_spmd(nc, in_maps, list(range(8)), trace=trace)
    global LAST_RES
    LAST_RES = res
    y = np.empty((B, TI, E), dtype=np.float32)
    for b in range(B):
        y[b] = res.results[2 * b]["y"] + res.results[2 * b + 1]["y"] + bu
    return y, res.exec_time_ns


def kernel(**inputs):
    y, _ = run(inputs, trace=False)
    return y
